# revision 30
# baseline (speedup 1.0000x reference)
"""Autoregressive GRU on 8 TRN2 NeuronCores.

Data-parallel: batch B=512 is split as 64 rows per core; the small GRU
weights are replicated and the T=128 sequential loop runs locally per core.

Key algebra (Keras GRU, reset_after=True, gate order [z, r, h]):
  step 0:  inp = 0, h = x  ->  gx = b[0], gh = x @ U + b[1]
  step t>=1: inp == h      ->  gx + gh uses (W + U) for the z and r gates
so per step ONE matmul stream against a host-prefused weight matrix:
  V  = [Wr+Ur | Uh | Wh | Wz+Uz]   (steps >= 1)   [D, 4D]
  V0 = [Ur   | Uh | 0  | Uz    ]   (step 0)       [D, 4D]
with PSUM banks streamed in order [r | z | hh | xh], then (u = 1 - z):
  r = sigmoid(rpre); z = sigmoid(zpre); u = sigmoid(-zpre)
  hhat = tanh(xh + r*hh);  h_new = z*h + u*hhat

Perf structure (rewrite of the f32r baseline, which ran 908us = 7.1us/step):
- fp16 weights + fp16 state: 16-bit matmuls stream 1 col/cycle like f32r
  (only fp8+DoubleRow is faster, but e4m3's 3 mantissa bits blow the 2e-2
  gate), yet fp16's 10 mantissa bits *improve* accuracy 5x over the old
  f32r-weights/bf16-state mix: rel err 2.1e-3 vs 9.6e-3 (bf16-everything
  would be 1.76e-2 - too close to the gate).  The win is structural:
- z AND u = sigmoid(+/-zpre) as two ACT ops (scale=-1 negates for free),
  so e = z*h is ONE off-chain DVE op and h_new = e + u*hhat.
- q = xh + p is a PE-accumulated identity matmul (I64^T @ p streamed into
  the xh PSUM bank, ~115ns) instead of a ~690ns DVE add with PSUM operand.
- h_new^T is assembled by PSUM ACCUMULATION of regular identity-moving
  matmuls (e^T chunks open each accumulator, m^T chunks close it): the
  f32 sum in PSUM IS the step output (copied f32 to SBUF and DMA'd in
  transposed layout; the host un-permutes at the end).  start=True
  clears the has_written bits of the WHOLE PSUM bank, so only the first
  MM per bank carries it.
- Everything after tanh is split into D-halves with SEPARATE tiles
  (dependency tracking is tile-granular): tanhA/tanhB on slices of the
  gX bank, mA/mB, 2+2 m^T matmuls, castA (ScalarE) || castB (VectorE)
  into split h^T stationaries hTa/hTb, so the next step's k=0,1 gate
  matmuls launch off castA while the B-half tail is still draining.
- Critical chain per step: castA -> r-MMs -> sigmoid(r) -> p=r*hh ->
  q-MM -> tanhA -> mA -> 2 mT-MMs -> castA.  Everything else (z, u, e,
  eT-MMs, B-half tail, h_new, f32 out copies, DMA) overlaps.
- HAM/P0 trap (measured, not theory): dependency-free filler matmuls get
  hoisted to t=0 by the scheduler; dependency-pinned fillers (zmov vs
  real stationary) DO hold the activity monitor at K=8/8 (147us of K=4/8
  time -> 12us) but tip the chip into the P0 power state - EVERY engine
  drops to 5/6 clock (tanh 682->817ns), a worse trade.  Splitting the
  xh bank into 8 N=256 matmuls did the same via extra LDWEIGHTS
  activity.  FILL=0 with the lean matmul schedule is the optimum: the
  HAM alternates warm(5.5us)/cold(7.6us) steps and that still beats
  both P0-locked variants.
"""

import numpy as np

B, D, T = 512, 512, 128
NCORES = 8
BLOC = B // NCORES  # 64
P = 128
KC = D // P  # 4 K-chunks
GW = 4 * D  # 2048 gate columns: [r | hh | xh | z] (host layout order)

_F16 = np.float16

# set by test harness to capture a profile; harmless when False
TRACE = False
TMPDIR = None
LAST = {}

# number of filler identity-MMs per stall gap (HAM warmth); tunable
FILL = 2


def _prepare_weights(W, U, b):
    """Host-side fusion. Returns (V, V0, bias) in math layout [r|hh|xh|z]."""
    Wz, Wr, Wh = W[:, :D], W[:, D : 2 * D], W[:, 2 * D :]
    Uz, Ur, Uh = U[:, :D], U[:, D : 2 * D], U[:, 2 * D :]
    V = np.concatenate([Wr + Ur, Uh, Wh, Wz + Uz], axis=1)  # [D, GW]
    V0 = np.concatenate([Ur, Uh, np.zeros_like(Wh), Uz], axis=1)
    b0, b1 = b[0], b[1]
    bias = np.concatenate(
        [b0[D : 2 * D] + b1[D : 2 * D], b1[2 * D :], b0[2 * D :], b0[:D] + b1[:D]]
    )  # [GW], order [r | hh | xh | z]
    return V, V0, bias


def _dev_layout(V):
    # V_dev[p, k*GW + j] = V[k*128 + p, j]
    return np.ascontiguousarray(
        V.reshape(KC, P, GW).transpose(1, 0, 2).reshape(P, KC * GW)
    )


_CACHE = {}


def _build(has_bias: bool):
    import concourse.mybir as mybir
    import concourse.tile as tile
    from concourse import bacc
    from concourse.masks import make_identity

    f32 = mybir.dt.float32
    f16 = mybir.dt.float16
    AF = mybir.ActivationFunctionType

    nc = bacc.Bacc(
        "TRN2", target_bir_lowering=False, debug=False, num_devices=NCORES
    )
    v0_d = nc.dram_tensor("v0", [P, KC * GW], f16, kind="ExternalInput").ap()
    v_d = nc.dram_tensor("v", [P, KC * GW], f16, kind="ExternalInput").ap()
    h0_d = nc.dram_tensor("h0", [BLOC, D], f16, kind="ExternalInput").ap()
    h0T_d = nc.dram_tensor("h0T", [P, KC * BLOC], f16, kind="ExternalInput").ap()
    if has_bias:
        bias_d = nc.dram_tensor("bias", [BLOC, GW], f32, kind="ExternalInput").ap()
    # transposed per-step output: out[t, p, k*BLOC + b] = h_t[b, k*P + p]
    out_d = nc.dram_tensor("out", [T, P, KC * BLOC], f32, kind="ExternalOutput").ap()

    with tile.TileContext(nc) as tc:
        with (
            tc.tile_pool(name="const", bufs=1) as cpool,
            tc.tile_pool(name="state", bufs=2) as spool,
            tc.tile_pool(name="work", bufs=2) as wpool,
            tc.tile_pool(name="outp", bufs=6) as opool,
            tc.tile_pool(name="gates", bufs=1, space="PSUM") as gpool,
            tc.tile_pool(name="trp", bufs=1, space="PSUM") as trpool,
            tc.tile_pool(name="scr", bufs=1, space="PSUM") as scrpool,
        ):
            v0_sb = cpool.tile([P, KC * GW], f16, tag="v0")
            v_sb = cpool.tile([P, KC * GW], f16, tag="v")
            ident = cpool.tile([BLOC, BLOC], f16, tag="ident")
            # zeroed moving operand for HAM-filler matmuls: streaming zeros
            # toggles no multiplier bits, so the filler keeps the activity
            # monitor fed WITHOUT the power draw that trips the P0 downclock
            zmov = cpool.tile([P, 512], f16, tag="zmov")
            nc.sync.dma_start(v0_sb[:], v0_d[:])
            make_identity(nc, ident[:])
            nc.gpsimd.memset(zmov[:], 0.0)

            h = spool.tile([BLOC, D], f16, tag="h")
            hTa = spool.tile([P, 2 * BLOC], f16, tag="hTa")
            hTb = spool.tile([P, 2 * BLOC], f16, tag="hTb")
            nc.sync.dma_start(h[:], h0_d[:])
            nc.sync.dma_start(hTa[:], h0T_d[:, 0 : 2 * BLOC])
            nc.sync.dma_start(hTb[:], h0T_d[:, 2 * BLOC :])
            nc.sync.dma_start(v_sb[:], v_d[:])
            if has_bias:
                bias_sb = cpool.tile([BLOC, GW], f32, tag="bias")
                nc.sync.dma_start(bias_sb[:], bias_d[:])

            # PE warm-up: regular matmuls that depend only on the
            # locally-built identity (not on any DMA) flip the HAM clock
            # gate to K=8/8 while the weight DMAs are still in flight.
            scr = scrpool.tile([BLOC, 512], f32, tag="scr", name="wu")
            for i in range(40):
                nc.tensor.matmul(
                    scr[:, :BLOC], ident[:], ident[:], start=True, stop=True
                )

            for t in range(T):
                vsb = v0_sb if t == 0 else v_sb
                last = t == T - 1
                # PSUM gate banks, accumulation order [r, z, hh, xhL, xhR];
                # the xh bank is two tiles so tanh-L's dependency closes at
                # the L-half q matmul instead of the full bank (dependency
                # tracking is tile-granular)
                gR = gpool.tile([BLOC, 512], f32, tag="gR", name="gR")
                gH = gpool.tile([BLOC, 512], f32, tag="gH", name="gH")
                gZ = gpool.tile([BLOC, 512], f32, tag="gZ", name="gZ")
                gX = gpool.tile([BLOC, 512], f32, tag="gX", name="gX")
                # h_new^T accumulators, split by D-half so the fp16 casts
                # and next-step stationaries gate at half granularity
                hTpA = trpool.tile([P, 2 * BLOC], f32, tag="hTpA", name="hTpA")
                hTpB = trpool.tile([P, 2 * BLOC], f32, tag="hTpB", name="hTpB")

                def stat(k):
                    # stationary for contraction chunk k: halves of h^T
                    src = hTa if k < 2 else hTb
                    kk = k % 2
                    return src[:, kk * BLOC : (kk + 1) * BLOC]

                def bank(g, n, stop=True, cols=None):
                    lo, hi = cols or (0, 512)
                    for k in range(KC):
                        nc.tensor.matmul(
                            g[:, lo:hi],
                            stat(k),
                            vsb[:, k * GW + n * 512 + lo : k * GW + n * 512 + hi],
                            start=(k == 0),
                            stop=(k == KC - 1) and stop,
                        )
                    if has_bias:
                        nc.vector.tensor_add(
                            g[:, lo:hi],
                            g[:, lo:hi],
                            bias_sb[:, n * 512 + lo : n * 512 + hi],
                        )

                bank(gR, 0)  # rpre
                r = wpool.tile([BLOC, D], f16, tag="r", name="r")
                nc.scalar.activation(r[:], gR[:], AF.Sigmoid)
                bank(gZ, 3)  # zpre
                z = wpool.tile([BLOC, D], f16, tag="z", name="z")
                nc.scalar.activation(z[:], gZ[:], AF.Sigmoid)
                u = wpool.tile([BLOC, D], f16, tag="u", name="u")
                nc.scalar.activation(u[:], gZ[:], AF.Sigmoid, scale=-1.0)
                bank(gH, 1)  # hh
                p = wpool.tile([BLOC, D], f16, tag="p", name="p")
                nc.vector.tensor_mul(p[:], r[:], gH[:])
                # e = z*h = h - u*h: single DVE op thanks to the extra
                # sigmoid; feeds the eT transposes well before the m path
                e = wpool.tile([BLOC, D], f16, tag="e", name="e")
                nc.vector.tensor_mul(e[:], z[:], h[:])
                bank(gX, 2, stop=False)  # xh (accumulation stays open for q)
                # q = xh + p via PE accumulation: I64^T @ p streamed into gX
                nc.tensor.matmul(gX[:], ident[:], p[:], start=False, stop=True)
                # e^T chunks open the h_new^T accumulation (regular MMs).
                # NB: start=True clears the has_written bits of the WHOLE
                # PSUM bank, so only the first MM targeting each bank may
                # carry it — a start on each chunk wipes the prior chunks.
                for k in range(KC):
                    hp = hTpA if k < 2 else hTpB
                    nc.tensor.matmul(
                        hp[:, (k % 2) * BLOC : (k % 2 + 1) * BLOC],
                        e[:, k * P : (k + 1) * P],
                        ident[:],
                        start=(k % 2 == 0),
                        stop=False,
                    )
                if not last:
                    for f in range(FILL):
                        nc.tensor.matmul(
                            scr[:], stat(f % KC), zmov[:], start=True, stop=True
                        )

                # tanh in two ACT ops over halves of the SAME bank: the
                # A-half result (and everything downstream of it) is ready
                # ~0.5us earlier than one full-width tanh would allow
                hhatA = wpool.tile([BLOC, D // 2], f16, tag="hhatA", name="hhatA")
                nc.scalar.activation(hhatA[:], gX[:, : D // 2], AF.Tanh)
                # m = u * hhat, split by D-half so the A-side transposes,
                # cast and next-step k=0,1 matmuls start before mB exists
                mA = wpool.tile([BLOC, D // 2], f16, tag="mA", name="mA")
                nc.vector.tensor_mul(mA[:], u[:, : D // 2], hhatA[:])
                for k in range(2):  # close A: hTpA = (e + m)^T left half
                    nc.tensor.matmul(
                        hTpA[:, k * BLOC : (k + 1) * BLOC],
                        mA[:, k * P : (k + 1) * P],
                        ident[:],
                        start=False,
                        stop=(k == 1),
                    )
                hhatB = wpool.tile([BLOC, D // 2], f16, tag="hhatB", name="hhatB")
                nc.scalar.activation(hhatB[:], gX[:, D // 2 :], AF.Tanh)
                mB = wpool.tile([BLOC, D // 2], f16, tag="mB", name="mB")
                nc.vector.tensor_mul(mB[:], u[:, D // 2 :], hhatB[:])
                if not last:
                    hTa_new = spool.tile([P, 2 * BLOC], f16, tag="hTa")
                    nc.scalar.copy(hTa_new[:], hTpA[:])
                for k in range(2):  # close B
                    nc.tensor.matmul(
                        hTpB[:, k * BLOC : (k + 1) * BLOC],
                        mB[:, k * P : (k + 1) * P],
                        ident[:],
                        start=False,
                        stop=(k == 1),
                    )
                if not last:
                    hTb_new = spool.tile([P, 2 * BLOC], f16, tag="hTb")
                    nc.vector.tensor_copy(hTb_new[:], hTpB[:])
                    hTa, hTb = hTa_new, hTb_new

                of = opool.tile([P, KC * BLOC], f32, tag="of", name="of")
                nc.vector.tensor_copy(of[:, : 2 * BLOC], hTpA[:])
                nc.vector.tensor_copy(of[:, 2 * BLOC :], hTpB[:])
                nc.sync.dma_start(out_d[t], of[:])

                if not last:
                    h_new = spool.tile([BLOC, D], f16, tag="h")
                    nc.vector.tensor_add(h_new[:, : D // 2], e[:, : D // 2], mA[:])
                    nc.vector.tensor_add(h_new[:, D // 2 :], e[:, D // 2 :], mB[:])
                    h = h_new

    nc.compile()
    return nc


def kernel(x, W, U, b):
    from concourse.bass_utils import run_bass_kernel_spmd

    x = np.asarray(x, dtype=np.float32)
    W = np.asarray(W, dtype=np.float32)
    U = np.asarray(U, dtype=np.float32)
    b = np.asarray(b, dtype=np.float32)

    V, V0, bias = _prepare_weights(W, U, b)
    has_bias = bool(np.any(bias != 0.0))
    v_dev = _dev_layout(V).astype(_F16)
    v0_dev = _dev_layout(V0).astype(_F16)

    key = ("gru", has_bias)
    if key not in _CACHE:
        _CACHE[key] = _build(has_bias)
    nc = _CACHE[key]

    in_maps = []
    for i in range(NCORES):
        xs = x[i * BLOC : (i + 1) * BLOC]  # [64, 512]
        xs16 = xs.astype(_F16)
        m = {
            "v0": v0_dev,
            "v": v_dev,
            "h0": xs16,
            "h0T": np.ascontiguousarray(
                xs16.reshape(BLOC, KC, P).transpose(2, 1, 0).reshape(P, KC * BLOC)
            ),
        }
        if has_bias:
            m["bias"] = np.ascontiguousarray(
                np.broadcast_to(bias[None, :], (BLOC, GW))
            ).astype(np.float32)
        in_maps.append(m)

    res = run_bass_kernel_spmd(
        nc, in_maps, core_ids=list(range(NCORES)), trace=TRACE, tmpdir=TMPDIR
    )
    LAST["exec_time_ns"] = res.exec_time_ns
    LAST["results"] = res
    # un-permute: out_dev[t, p, k*BLOC+b] = h_t[b, k*P+p]
    parts = []
    for i in range(NCORES):
        arr = res.results[i]["out"]  # [T, P, KC*BLOC] f32
        arr = arr.reshape(T, P, KC, BLOC).transpose(3, 0, 2, 1).reshape(BLOC, T, D)
        parts.append(arr)
    return np.ascontiguousarray(np.concatenate(parts, axis=0)).astype(np.float32)
